# revision 63
# baseline (speedup 1.0000x reference)
"""GAT layer (nn_GAT) on 8 Trainium2 NeuronCores — Bass/Tile SPMD kernel.

Math (per head h):
    Wh   = x @ W[h]                         [N, HID]
    s_i  = Wh_i . a1[h],  d_j = Wh_j . a2[h]
    e_ij = leakyrelu(s_i + d_j, 0.2), masked by adj, softmax over j
    out  = elu(att @ Wh)

Restructuring used on-device (the key trick):
    exp(LR(z)) = max(e^z, e^{az})                      (a = 0.2 < 1)
    exp(LR(s_i + d_j)) = e^{a s_i} * max(w_i * A_j, B_j)
        with w = e^{(1-a)s}, A = e^{d}, B = e^{a d}
    The row factor e^{a s_i} cancels in the softmax, so the masked
    numerator is   p_ji = adjT_ji * max(w_i A_j, B_j)   — ONE fused
    TensorScalar (mult + max, both per-partition operands) and ONE
    tensor_tensor mask multiply per tile.  numerator and denominator
    come out of a single PE matmul with rhs = [Wh | 1].
    Everything is scaled by 2^-10 (cancels in the softmax ratio) so the
    fp16 N^2 path cannot overflow.

Schedule (v2): the N^2 mask work is the DVE bottleneck and the input DMA
(17MB) covers the first ~45us, so:
  - inputs are DMA'd in interleaved stripes (x column-stripes + adjacency
    groups) so projection AND attention start a few us in;
  - attention runs pair-major across TWO resident heads at a time (their
    PSUM accumulators live simultaneously: 4 banks + 4 projection banks),
    interleaved into the projection loop at the data-arrival rate; heads
    2,3 reuse the banks once heads 0,1 retire;
  - ~40% of the mask tensor_tensor ops run on the otherwise-idle GPSIMD
    engine (DVE and GPSIMD produce p-tiles concurrently);
  - dummy identity matmuls at t=0 keep the PE activity monitor from
    starting the real work at half clock.

elu(v) = relu(v) + min(exp(v), 1) - 1.
"""

from contextlib import ExitStack

import numpy as np

import concourse.bass as bass
import concourse.bacc as bacc
import concourse.mybir as mybir
import concourse.tile as tile
from concourse.bass_utils import run_bass_kernel_spmd
from concourse.masks import make_identity

N, F, HID, H = 4096, 512, 64, 8
ALPHA = 0.2
HG, RG = 2, 4                 # head groups x row groups
HPC, RPC = H // HG, N // RG   # 4 heads / 1024 rows per core
NB = N // 128                 # 32 projection row-blocks == j-chunks
FB = F // 128                 # 4 contraction chunks
IB = RPC // 128               # 8 output row-blocks per core
NCORES = 8
NPAIR = NB // 2               # 16 j-chunk pairs per head
NSTRIPE = 8                   # x DMA stripes (512 cols each)
SC_BIAS = -float(np.log(1024.0))  # exp(z + SC_BIAS) = exp(z) / 1024
WARMUP_MM = 40                # dummy matmuls to pre-warm the PE clock gate
                              # (more is NOT better: 110+ delayed the first
                              # projection and one run produced bad output)

_CACHE = {}


def _build():
    f16, f32 = mybir.dt.float16, mybir.dt.float32
    Alu = mybir.AluOpType
    Act = mybir.ActivationFunctionType

    nc = bacc.Bacc()
    # all inputs are host-swizzled so each SBUF partition reads one long
    # contiguous DRAM run (big DMA descriptors)
    xhi = nc.declare_dram_parameter("xhi", [128, FB, N], f16, isOutput=False)
    xlo = nc.declare_dram_parameter("xlo", [128, FB, N], f16, isOutput=False)
    # [ W(4 heads, 64 each) | wsd_hi(8) | wsd_lo(8) ]
    wext = nc.declare_dram_parameter("wext", [128, FB, 272], f16, isOutput=False)
    adjt = nc.declare_dram_parameter("adjt", [128, NB, RPC], f16, isOutput=False)
    # partition-major output: [h, p, ib, d] so each head is ONE DMA with 2KB
    # per-partition runs (the [h, i, d] layout needed 8 DMAs x 256B runs,
    # ~1.3us of descriptor generation each); the host unscrambles
    out_t = nc.declare_dram_parameter("out", [HPC, 128, IB, HID], f32, isOutput=True)

    with ExitStack() as ctx:
        tc = ctx.enter_context(tile.TileContext(nc))
        const = ctx.enter_context(tc.tile_pool(name="const", bufs=1))
        ppool = ctx.enter_context(tc.tile_pool(name="ppool", bufs=2))
        epool = ctx.enter_context(tc.tile_pool(name="epool", bufs=3))
        opool = ctx.enter_context(tc.tile_pool(name="opool", bufs=2))

        wext_sb = const.tile([128, FB, 272], f16)
        xhi_sb = const.tile([128, FB, N], f16)
        xlo_sb = const.tile([128, FB, N], f16)
        adjt_tiles = [
            const.tile([128, 4, RPC], f16, name=f"adjt{g}") for g in range(8)
        ]
        rhs_sb = const.tile([128, HPC, NB, 65], f16)
        sd_sb = const.tile([128, NB, 8], f32)
        A_sb = const.tile([128, NB, HPC], f32)
        B_sb = const.tile([128, NB, HPC], f32)
        nB_sb = const.tile([128, NB, HPC], f32)
        wcol_sb = const.tile([128, IB, HPC], f16)
        ident_sb = const.tile([128, 128], f16)
        wct_sb = const.tile([32, 128], f16)
        wbc4 = const.tile([128, HPC, RPC], f16)
        scbias = const.tile([128, 1], f32)
        nc.vector.memset(scbias, SC_BIAS)
        make_identity(nc, ident_sb[:, :])
        dram = ctx.enter_context(tc.tile_pool(name="dram", bufs=1, space="DRAM"))
        # DRAM bounce buffer for the w broadcast: [h, b, i] so each head's
        # 1024 w values are one contiguous 2KB run
        w4_d = dram.tile([HPC, IB, 128], f16)

        # DMA issue order: HWDGE queues drain roughly in issue order.  The
        # first ~45us are DMA-bound, so interleave x column-stripes
        # (projection food) with adjacency groups (attention food): stripe s
        # unlocks projection row-blocks 4s..4s+3; adjacency group s unlocks
        # pairs 2s, 2s+1 for every head.
        nc.sync.dma_start(out=wext_sb[:, :, :], in_=wext[:, :, :])
        SW = N // NSTRIPE  # 512 cols per stripe
        # stripes 0,1 first (they gate the w factors via projection blocks
        # 0..7), then adjacency groups woven between the remaining stripes
        order = [("x", 0), ("x", 1), ("a", 0)]
        for s in range(2, NSTRIPE):
            order += [("x", s), ("a", s - 1)]
        order.append(("a", NSTRIPE - 1))
        for kind, s in order:
            if kind == "x":
                c0, c1 = s * SW, (s + 1) * SW
                nc.sync.dma_start(out=xhi_sb[:, :, c0:c1], in_=xhi[:, :, c0:c1])
                nc.sync.dma_start(out=xlo_sb[:, :, c0:c1], in_=xlo[:, :, c0:c1])
            else:
                nc.sync.dma_start(
                    out=adjt_tiles[s][:, :, :], in_=adjt[:, 4 * s:4 * (s + 1), :]
                )
        # softmax-denominator ones column of the matmul rhs
        nc.vector.memset(rhs_sb[:, :, :, 64:65], 1.0)

        # ---- PSUM pools: 4 attention banks (two resident heads) + 4
        # projection banks (pw x2, bcps, wctps) = 8 exactly.
        psmain = ctx.enter_context(tc.tile_pool(name="psmain", bufs=1, space="PSUM"))
        pswide = tc.alloc_tile_pool(name="pswide", bufs=3, space="PSUM")

        def alloc_head_psums(h):
            ab = "ab"[h % 2]
            P0 = psmain.tile([128, 4, 65], f32, tag=f"P0{ab}", name=f"P0_{h}")
            P1 = psmain.tile([128, 4, 65], f32, tag=f"P1{ab}", name=f"P1_{h}")
            return P0, P1

        # ---- PE warm-up: keep the HAM activity monitor busy from t~0 so the
        # first real matmuls run at full clock.  Results are discarded (the
        # warm tile is a regular pw rotation slot).
        warm_ps = pswide.tile([128, 264], f32, tag="pw")
        for k in range(WARMUP_MM):
            nc.tensor.matmul(
                warm_ps[:, 0:128], ident_sb[:, :], ident_sb[:, :],
                start=(k == 0), stop=(k == WARMUP_MM - 1),
                skip_group_check=True,
            )

        def emit_quad(h, jc0, psums):
            # four j-chunks per mask op (one full adjacency tile): TS scalars
            # differ per chunk so the fused mult+max stays per-chunk; the mask
            # (elementwise min with adj stored as {0, 60000}) runs once over
            # [128, 4*RPC], amortizing the per-op DVE init/drain overhead.
            P0, P1 = psums
            g = jc0 // 4
            t = ppool.tile([128, 4, RPC], f16, tag="t", name=f"t_{h}_{jc0}")
            for q in range(4):
                jc = jc0 + q
                nc.vector.tensor_scalar(
                    t[:, q, :], wbc4[:, h, :],
                    A_sb[:, jc, h:h + 1], B_sb[:, jc, h:h + 1],
                    Alu.mult, Alu.max,
                )
            p = ppool.tile([128, 4, RPC], f16, tag="p", name=f"p_{h}_{jc0}")
            nc.vector.tensor_tensor(
                p[:, :, :], t[:, :, :], adjt_tiles[g][:, :, :], Alu.min
            )
            for q in range(4):
                jc = jc0 + q
                for ib in range(IB):
                    P = P0 if ib < 4 else P1
                    nc.tensor.matmul(
                        P[:, ib % 4, :], p[:, q, ib * 128:(ib + 1) * 128],
                        rhs_sb[:, h, jc, :],
                        start=(jc == 0 and ib % 4 == 0),
                        stop=(jc == NB - 1 and ib % 4 == 3),
                        skip_group_check=True,
                    )

        def emit_epilogue(h, psums, split=False):
            P0, P1 = psums
            osb = opool.tile([128, IB, HID], f32, tag="osb", name=f"osb{h}")
            # batched reciprocal of the four denominator columns per P-tile
            rcps = []
            for half, Ph in enumerate((P0, P1)):
                rcp = epool.tile([128, 4, 1], f32, tag="rcp", name=f"rcp{h}_{half}")
                nc.vector.reciprocal(rcp, Ph[:, :, 64:65])
                rcps.append(rcp)
            for ib in range(IB):
                P = (P0 if ib < 4 else P1)[:, ib % 4, :]
                rcp = rcps[ib // 4][:, ib % 4, :]
                rt = epool.tile([128, 64], f32, tag="rt", name=f"rt{h}_{ib}")
                if split:
                    # final head: DVE is idle by now — run the relu branch
                    # there so the ACT chain (the tail critical path) halves
                    nc.vector.tensor_scalar(
                        rt, P[:, 0:64], rcp, 0.0, Alu.mult, Alu.max
                    )
                else:
                    nc.scalar.activation(rt, P[:, 0:64], Act.Relu, scale=rcp)
                qt = epool.tile([128, 64], f32, tag="qt", name=f"qt{h}_{ib}")
                nc.scalar.activation(qt, P[:, 0:64], Act.Exp, scale=rcp)
                # elu(v) = relu(v) - relu(1 - e^v)
                mt = epool.tile([128, 64], f32, tag="mt", name=f"mt{h}_{ib}")
                nc.scalar.activation(mt, qt, Act.Relu, scale=-1.0, bias=1.0)
                nc.vector.tensor_tensor(osb[:, ib, :], rt, mt, Alu.subtract)
            nc.sync.dma_start(out=out_t[h, :, :, :], in_=osb)

        def emit_wbc(h):
            # broadcast head h's w row to all 128 partitions: DRAM bounce
            # read with a 0-stride partition dim on the scalar HWDGE queue
            # (per-head reads so head 0's mask stream starts on the first;
            # GPSIMD partition_broadcast can't source partitions > 0)
            nc.scalar.dma_start(
                out=wbc4[:, h, :],
                in_=w4_d[h, :, :].partition_broadcast(128),
            )

        # ---- quad schedule ----
        # Phase 1 (inside the projection loop): heads 0,1 quad-major, emitted
        # at the adjacency/projection arrival rate so the in-order engine
        # queues interleave projection and attention work.  Phase 2: heads
        # 2,3, h-major so head 2's epilogue overlaps head 3's quads.
        NQUAD = NB // 4
        ps01 = [None, None]
        phase1 = [(q, h) for q in range(NQUAD) for h in range(2)]
        emitted = [0]

        def emit_ready_quads(nb_done):
            # rhs chunk jc comes from projection block nb==jc, so quad q
            # needs nb >= 4q+3; stay a chunk behind to avoid head-of-line
            # blocking in the in-order PE queue.
            while emitted[0] < len(phase1):
                q, h = phase1[emitted[0]]
                if 4 * q + 4 > nb_done:
                    return
                emit_quad(h, 4 * q, ps01[h])
                emitted[0] += 1

        # ---- projection loop ----
        for nb in range(NB):
            pw = pswide.tile([128, 264], f32)
            # merged-stationary order: each xhi f-chunk is loaded once and
            # used for both the wide W|wsd_hi stream and the wsd_lo column
            # accumulation; then the xlo chunks add the last double-fp16 term.
            for f in range(FB):
                nc.tensor.matmul(
                    pw, xhi_sb[:, f, nb * 128:(nb + 1) * 128], wext_sb[:, f, 0:264],
                    start=(f == 0), stop=False,
                )
                nc.tensor.matmul(
                    pw[:, 256:264], xhi_sb[:, f, nb * 128:(nb + 1) * 128],
                    wext_sb[:, f, 264:272], start=False, stop=False,
                )
            for f in range(FB):
                nc.tensor.matmul(
                    pw[:, 256:264], xlo_sb[:, f, nb * 128:(nb + 1) * 128],
                    wext_sb[:, f, 256:264], start=False, stop=(f == FB - 1),
                )
            # sd first (the w/A/B chain is on the critical path), then rhs;
            # the own-row sd copies get scheduler priority so wcol/wct isn't
            # stuck behind earlier blocks' A/B/rhs copies in the ACT queue
            if nb < IB:
                with tc.high_priority():
                    nc.scalar.activation(
                        sd_sb[:, nb, :], pw[:, 256:264], Act.Copy
                    )
            else:
                nc.scalar.activation(sd_sb[:, nb, :], pw[:, 256:264], Act.Copy)
            dcols = sd_sb[:, nb:nb + 1, 1:8:2]
            nc.scalar.activation(A_sb[:, nb:nb + 1, :], dcols, Act.Exp)
            nc.scalar.activation(
                B_sb[:, nb:nb + 1, :], dcols, Act.Exp,
                scale=ALPHA, bias=scbias[:, :],
            )
            nc.scalar.activation(
                rhs_sb[:, :, nb, 0:64],
                pw[:, 0:256].rearrange("p (h d) -> p h d", h=HPC),
                Act.Copy,
            )
            if nb == IB - 1:
                # own rows (blocks 0..7 thanks to the permutation): w factors,
                # transpose to one free-dim row — no DMA (a DMA here starves
                # behind the bulk input stream).  The whole chain runs at
                # scheduler priority 0 so the ACT queue doesn't interleave
                # later blocks' copies into it (it gates every mask op).
                with tc.high_priority():
                    nc.scalar.activation(
                        wcol_sb[:, :, :], sd_sb[:, 0:IB, 0:8:2],
                        Act.Exp, scale=1.0 - ALPHA, bias=scbias[:, :],
                    )
                    wct_ps = pswide.tile([32, 128], f16, tag="wctps", bufs=1)
                    nc.tensor.transpose(
                        wct_ps, wcol_sb[:, :, :].rearrange("p a b -> p (a b)"),
                        ident_sb[:, :],
                    )
                    nc.scalar.activation(wct_sb[:, :], wct_ps[:, :], Act.Copy)
                    # bounce w through DRAM (partition-major -> head-major)
                    # on the scalar engine's HWDGE queue
                    nc.scalar.dma_start(
                        out=w4_d.rearrange("h b i -> b h i"), in_=wct_sb[:, :]
                    )
                    for h in range(HPC):
                        emit_wbc(h)
                ps01[0] = alloc_head_psums(0)
                ps01[1] = alloc_head_psums(1)
            if nb > IB:
                emit_ready_quads(nb)
        # drain leftovers h-major so each head's epilogue overlaps the next
        # head's remaining quads
        leftover = phase1[emitted[0]:]
        for hh in range(2):
            for q, h in leftover:
                if h == hh:
                    emit_quad(h, 4 * q, ps01[h])
            emit_epilogue(hh, ps01[hh])
        pswide.release()

        # ---- phase 2: heads 2,3 reuse the released accumulator banks ----
        ps23 = [alloc_head_psums(2), alloc_head_psums(3)]
        for q in range(NQUAD):
            emit_quad(2, 4 * q, ps23[0])
        emit_epilogue(2, ps23[0])
        for q in range(NQUAD):
            emit_quad(3, 4 * q, ps23[1])
        emit_epilogue(3, ps23[1], split=True)
    nc.finalize()
    return nc


def _get_nc():
    if "nc" not in _CACHE:
        _CACHE["nc"] = _build()
    return _CACHE["nc"]


def _prepare_in_maps(x, adj, W, a):
    x = np.asarray(x, np.float32)
    adj = np.asarray(adj, np.float32)
    W = np.asarray(W, np.float32)
    a = np.asarray(a, np.float32)
    xT = np.ascontiguousarray(x.T)
    adjT = np.ascontiguousarray(adj.T)
    all_rows = np.arange(N)
    in_maps = []
    for c in range(NCORES):
        hg, rg = divmod(c, RG)
        own = np.arange(rg * RPC, (rg + 1) * RPC)
        perm = np.concatenate([own, np.delete(all_rows, own)])
        xt = xT[:, perm]
        xhi = xt.astype(np.float16)
        xlo = (xt - xhi.astype(np.float32)).astype(np.float16)
        heads = [hg * HPC + h for h in range(HPC)]
        wsd = np.stack(
            sum([[W[gh] @ a[gh, :HID], W[gh] @ a[gh, HID:]] for gh in heads], []),
            axis=1,
        ).astype(np.float32)  # [F, 8] cols (h0 s, h0 d, h1 s, ...)
        wsdh = wsd.astype(np.float16)
        wsdl = (wsd - wsdh.astype(np.float32)).astype(np.float16)
        wext = np.concatenate(
            [W[gh] for gh in heads] + [wsdh, wsdl], axis=1
        ).astype(np.float16)  # [F, 272]
        adjt_c = (adjT[perm][:, own] * 60000.0).astype(np.float16)

        def swz(m):
            # [K*128, M] -> [128, K, M]: partition-major so each SBUF
            # partition reads one contiguous DRAM run
            k = m.shape[0] // 128
            return np.ascontiguousarray(
                m.reshape(k, 128, m.shape[1]).transpose(1, 0, 2)
            )

        in_maps.append({
            "xhi": swz(xhi),
            "xlo": swz(xlo),
            "wext": swz(wext),
            "adjt": swz(adjt_c),
        })
    return in_maps


def _assemble(results):
    full = np.empty((N, H * HID), np.float32)
    for c in range(NCORES):
        hg, rg = divmod(c, RG)
        o = results[c]["out"]  # [HPC, 128, IB, HID] partition-major
        o = o.transpose(0, 2, 1, 3).reshape(HPC, RPC, HID)
        full[rg * RPC:(rg + 1) * RPC, hg * HPC * HID:(hg + 1) * HPC * HID] = (
            o.transpose(1, 0, 2).reshape(RPC, HPC * HID)
        )
    return full


def _run(in_maps, **kw):
    return run_bass_kernel_spmd(_get_nc(), in_maps, list(range(NCORES)), **kw)


def kernel(x, adj, W, a):
    in_maps = _prepare_in_maps(x, adj, W, a)
    res = _run(in_maps)
    return _assemble(res.results)


# revision 65
# speedup vs baseline: 1.1590x; 1.1590x over previous
"""GAT layer (nn_GAT) on 8 Trainium2 NeuronCores — Bass/Tile SPMD kernel.

Math (per head h):
    Wh   = x @ W[h]                         [N, HID]
    s_i  = Wh_i . a1[h],  d_j = Wh_j . a2[h]
    e_ij = leakyrelu(s_i + d_j, 0.2), masked by adj, softmax over j
    out  = elu(att @ Wh)

Restructuring used on-device (the key trick):
    exp(LR(z)) = max(e^z, e^{az})                      (a = 0.2 < 1)
    exp(LR(s_i + d_j)) = e^{a s_i} * max(w_i * A_j, B_j)
        with w = e^{(1-a)s}, A = e^{d}, B = e^{a d}
    The row factor e^{a s_i} cancels in the softmax, so the masked
    numerator is   p_ji = adjT_ji * max(w_i A_j, B_j)   — ONE fused
    TensorScalar (mult + max, both per-partition operands) and ONE
    tensor_tensor mask multiply per tile.  numerator and denominator
    come out of a single PE matmul with rhs = [Wh | 1].
    Everything is scaled by 2^-10 (cancels in the softmax ratio) so the
    fp16 N^2 path cannot overflow.

Schedule (v2): the N^2 mask work is the DVE bottleneck and the input DMA
(17MB) covers the first ~45us, so:
  - inputs are DMA'd in interleaved stripes (x column-stripes + adjacency
    groups) so projection AND attention start a few us in;
  - attention runs pair-major across TWO resident heads at a time (their
    PSUM accumulators live simultaneously: 4 banks + 4 projection banks),
    interleaved into the projection loop at the data-arrival rate; heads
    2,3 reuse the banks once heads 0,1 retire;
  - ~40% of the mask tensor_tensor ops run on the otherwise-idle GPSIMD
    engine (DVE and GPSIMD produce p-tiles concurrently);
  - dummy identity matmuls at t=0 keep the PE activity monitor from
    starting the real work at half clock.

elu(v) = relu(v) + min(exp(v), 1) - 1.
"""

from contextlib import ExitStack

import numpy as np

import concourse.bass as bass
import concourse.bacc as bacc
import concourse.mybir as mybir
import concourse.tile as tile
from concourse.bass_utils import run_bass_kernel_spmd
from concourse.masks import make_identity

N, F, HID, H = 4096, 512, 64, 8
ALPHA = 0.2
HG, RG = 2, 4                 # head groups x row groups
HPC, RPC = H // HG, N // RG   # 4 heads / 1024 rows per core
NB = N // 128                 # 32 projection row-blocks == j-chunks
FB = F // 128                 # 4 contraction chunks
IB = RPC // 128               # 8 output row-blocks per core
NCORES = 8
NPAIR = NB // 2               # 16 j-chunk pairs per head
NSTRIPE = 8                   # x DMA stripes (512 cols each)
SC_BIAS = -float(np.log(1024.0))  # exp(z + SC_BIAS) = exp(z) / 1024
WARMUP_MM = 40                # dummy matmuls to pre-warm the PE clock gate
                              # (more is NOT better: 110+ delayed the first
                              # projection and one run produced bad output)

_CACHE = {}


def _build():
    f16, f32 = mybir.dt.float16, mybir.dt.float32
    Alu = mybir.AluOpType
    Act = mybir.ActivationFunctionType

    nc = bacc.Bacc()
    # all inputs are host-swizzled so each SBUF partition reads one long
    # contiguous DRAM run (big DMA descriptors)
    xhi = nc.declare_dram_parameter("xhi", [128, FB, N], f16, isOutput=False)
    xlo = nc.declare_dram_parameter("xlo", [128, FB, N], f16, isOutput=False)
    # [ W(4 heads, 64 each) | wsd_hi(8) | wsd_lo(8) ]
    wext = nc.declare_dram_parameter("wext", [128, FB, 272], f16, isOutput=False)
    adjt = nc.declare_dram_parameter("adjt", [128, NB, RPC], f16, isOutput=False)
    # partition-major output: [h, p, ib, d] so each head is ONE DMA with 2KB
    # per-partition runs (the [h, i, d] layout needed 8 DMAs x 256B runs,
    # ~1.3us of descriptor generation each); the host unscrambles
    out_t = nc.declare_dram_parameter("out", [HPC, 128, IB, HID], f32, isOutput=True)

    with ExitStack() as ctx:
        tc = ctx.enter_context(tile.TileContext(nc))
        const = ctx.enter_context(tc.tile_pool(name="const", bufs=1))
        ppool = ctx.enter_context(tc.tile_pool(name="ppool", bufs=2))
        epool = ctx.enter_context(tc.tile_pool(name="epool", bufs=3))
        opool = ctx.enter_context(tc.tile_pool(name="opool", bufs=2))

        wext_sb = const.tile([128, FB, 272], f16)
        xhi_sb = const.tile([128, FB, N], f16)
        xlo_sb = const.tile([128, FB, N], f16)
        adjt_tiles = [
            const.tile([128, 4, RPC], f16, name=f"adjt{g}") for g in range(8)
        ]
        rhs_sb = const.tile([128, HPC, NB, 65], f16)
        sd_sb = const.tile([128, NB, 8], f32)
        A_sb = const.tile([128, NB, HPC], f32)
        B_sb = const.tile([128, NB, HPC], f32)
        nB_sb = const.tile([128, NB, HPC], f32)
        wcol_sb = const.tile([128, IB, HPC], f16)
        ident_sb = const.tile([128, 128], f16)
        wct_sb = const.tile([32, 128], f16)
        wbc4 = const.tile([128, HPC, RPC], f16)
        scbias = const.tile([128, 1], f32)
        nc.vector.memset(scbias, SC_BIAS)
        make_identity(nc, ident_sb[:, :])
        dram = ctx.enter_context(tc.tile_pool(name="dram", bufs=1, space="DRAM"))
        # DRAM bounce buffer for the w broadcast: [h, b, i] so each head's
        # 1024 w values are one contiguous 2KB run
        w4_d = dram.tile([HPC, IB, 128], f16)

        # DMA issue order: HWDGE queues drain roughly in issue order.  The
        # first ~45us are DMA-bound, so interleave x column-stripes
        # (projection food) with adjacency groups (attention food): stripe s
        # unlocks projection row-blocks 4s..4s+3; adjacency group s unlocks
        # pairs 2s, 2s+1 for every head.
        nc.sync.dma_start(out=wext_sb[:, :, :], in_=wext[:, :, :])
        SW = N // NSTRIPE  # 512 cols per stripe
        # stripes 0,1 first (they gate the w factors via projection blocks
        # 0..7), then adjacency groups woven between the remaining stripes
        order = [("x", 0), ("x", 1), ("a", 0)]
        for s in range(2, NSTRIPE):
            order += [("x", s), ("a", s - 1)]
        order.append(("a", NSTRIPE - 1))
        for kind, s in order:
            if kind == "x":
                c0, c1 = s * SW, (s + 1) * SW
                nc.sync.dma_start(out=xhi_sb[:, :, c0:c1], in_=xhi[:, :, c0:c1])
                nc.sync.dma_start(out=xlo_sb[:, :, c0:c1], in_=xlo[:, :, c0:c1])
            else:
                nc.sync.dma_start(
                    out=adjt_tiles[s][:, :, :], in_=adjt[:, 4 * s:4 * (s + 1), :]
                )
        # softmax-denominator ones column of the matmul rhs
        nc.vector.memset(rhs_sb[:, :, :, 64:65], 1.0)

        # ---- PSUM pools: 4 attention banks (two resident heads) + 4
        # projection banks (pw x2, bcps, wctps) = 8 exactly.
        psmain = ctx.enter_context(tc.tile_pool(name="psmain", bufs=1, space="PSUM"))
        pswide = tc.alloc_tile_pool(name="pswide", bufs=3, space="PSUM")

        def alloc_head_psums(h):
            ab = "ab"[h % 2]
            P0 = psmain.tile([128, 4, 65], f32, tag=f"P0{ab}", name=f"P0_{h}")
            P1 = psmain.tile([128, 4, 65], f32, tag=f"P1{ab}", name=f"P1_{h}")
            return P0, P1

        # ---- PE warm-up: keep the HAM activity monitor busy from t~0 so the
        # first real matmuls run at full clock.  Results are discarded (the
        # warm tile is a regular pw rotation slot).
        warm_ps = pswide.tile([128, 264], f32, tag="pw")
        for k in range(WARMUP_MM):
            nc.tensor.matmul(
                warm_ps[:, 0:128], ident_sb[:, :], ident_sb[:, :],
                start=(k == 0), stop=(k == WARMUP_MM - 1),
                skip_group_check=True,
            )

        def emit_quad(h, jc0, psums):
            # four j-chunks per mask op (one full adjacency tile): TS scalars
            # differ per chunk so the fused mult+max stays per-chunk; the mask
            # (elementwise min with adj stored as {0, 60000}) runs once over
            # [128, 4*RPC], amortizing the per-op DVE init/drain overhead.
            P0, P1 = psums
            g = jc0 // 4
            t = ppool.tile([128, 4, RPC], f16, tag="t", name=f"t_{h}_{jc0}")
            for q in range(4):
                jc = jc0 + q
                nc.vector.tensor_scalar(
                    t[:, q, :], wbc4[:, h, :],
                    A_sb[:, jc, h:h + 1], B_sb[:, jc, h:h + 1],
                    Alu.mult, Alu.max,
                )
            p = ppool.tile([128, 4, RPC], f16, tag="p", name=f"p_{h}_{jc0}")
            nc.vector.tensor_tensor(
                p[:, :, :], t[:, :, :], adjt_tiles[g][:, :, :], Alu.min
            )
            for q in range(4):
                jc = jc0 + q
                for ib in range(IB):
                    P = P0 if ib < 4 else P1
                    nc.tensor.matmul(
                        P[:, ib % 4, :], p[:, q, ib * 128:(ib + 1) * 128],
                        rhs_sb[:, h, jc, :],
                        start=(jc == 0 and ib % 4 == 0),
                        stop=(jc == NB - 1 and ib % 4 == 3),
                        skip_group_check=True,
                    )

        def emit_epilogue(h, psums, split=False):
            P0, P1 = psums
            osb = opool.tile([128, IB, HID], f32, tag="osb", name=f"osb{h}")
            # batched reciprocal of the four denominator columns per P-tile
            rcps = []
            for half, Ph in enumerate((P0, P1)):
                rcp = epool.tile([128, 4, 1], f32, tag="rcp", name=f"rcp{h}_{half}")
                nc.vector.reciprocal(rcp, Ph[:, :, 64:65])
                rcps.append(rcp)
            for ib in range(IB):
                P = (P0 if ib < 4 else P1)[:, ib % 4, :]
                rcp = rcps[ib // 4][:, ib % 4, :]
                rt = epool.tile([128, 64], f32, tag="rt", name=f"rt{h}_{ib}")
                if split:
                    # final head: DVE is idle by now — run the relu branch
                    # there so the ACT chain (the tail critical path) halves
                    nc.vector.tensor_scalar(
                        rt, P[:, 0:64], rcp, 0.0, Alu.mult, Alu.max
                    )
                else:
                    nc.scalar.activation(rt, P[:, 0:64], Act.Relu, scale=rcp)
                qt = epool.tile([128, 64], f32, tag="qt", name=f"qt{h}_{ib}")
                nc.scalar.activation(qt, P[:, 0:64], Act.Exp, scale=rcp)
                # elu(v) = relu(v) - relu(1 - e^v)
                mt = epool.tile([128, 64], f32, tag="mt", name=f"mt{h}_{ib}")
                nc.scalar.activation(mt, qt, Act.Relu, scale=-1.0, bias=1.0)
                nc.vector.tensor_tensor(osb[:, ib, :], rt, mt, Alu.subtract)
            nc.sync.dma_start(out=out_t[h, :, :, :], in_=osb)

        def emit_wbc(h):
            # broadcast head h's w row to all 128 partitions: DRAM bounce
            # read with a 0-stride partition dim on the scalar HWDGE queue
            # (per-head reads so head 0's mask stream starts on the first;
            # GPSIMD partition_broadcast can't source partitions > 0)
            nc.scalar.dma_start(
                out=wbc4[:, h, :],
                in_=w4_d[h, :, :].partition_broadcast(128),
            )

        # ---- quad schedule ----
        # Phase 1 (inside the projection loop): heads 0,1 quad-major, emitted
        # at the adjacency/projection arrival rate so the in-order engine
        # queues interleave projection and attention work.  Phase 2: heads
        # 2,3, h-major so head 2's epilogue overlaps head 3's quads.
        NQUAD = NB // 4
        ps01 = [None, None]
        phase1 = [(q, h) for q in range(NQUAD) for h in range(2)]
        emitted = [0]

        def emit_ready_quads(nb_done):
            # rhs chunk jc comes from projection block nb==jc, so quad q
            # needs nb >= 4q+3; stay a chunk behind to avoid head-of-line
            # blocking in the in-order PE queue.
            while emitted[0] < len(phase1):
                q, h = phase1[emitted[0]]
                if 4 * q + 4 > nb_done:
                    return
                emit_quad(h, 4 * q, ps01[h])
                emitted[0] += 1

        # ---- projection loop ----
        for nb in range(NB):
            pw = pswide.tile([128, 264], f32)
            # merged-stationary order: each xhi f-chunk is loaded once and
            # used for both the wide W|wsd_hi stream and the wsd_lo column
            # accumulation; then the xlo chunks add the last double-fp16 term.
            for f in range(FB):
                nc.tensor.matmul(
                    pw, xhi_sb[:, f, nb * 128:(nb + 1) * 128], wext_sb[:, f, 0:264],
                    start=(f == 0), stop=False,
                )
                nc.tensor.matmul(
                    pw[:, 256:264], xhi_sb[:, f, nb * 128:(nb + 1) * 128],
                    wext_sb[:, f, 264:272], start=False, stop=False,
                )
            for f in range(FB):
                nc.tensor.matmul(
                    pw[:, 256:264], xlo_sb[:, f, nb * 128:(nb + 1) * 128],
                    wext_sb[:, f, 256:264], start=False, stop=(f == FB - 1),
                )
            # sd first (the w/A/B chain is on the critical path), then rhs.
            # (Scheduler high_priority hints here and on the wct chain were
            # tried and cost +28us — head-of-line blocking in the in-order
            # engine queues.)
            nc.scalar.activation(sd_sb[:, nb, :], pw[:, 256:264], Act.Copy)
            dcols = sd_sb[:, nb:nb + 1, 1:8:2]
            nc.scalar.activation(A_sb[:, nb:nb + 1, :], dcols, Act.Exp)
            nc.scalar.activation(
                B_sb[:, nb:nb + 1, :], dcols, Act.Exp,
                scale=ALPHA, bias=scbias[:, :],
            )
            nc.scalar.activation(
                rhs_sb[:, :, nb, 0:64],
                pw[:, 0:256].rearrange("p (h d) -> p h d", h=HPC),
                Act.Copy,
            )
            if nb == IB - 1:
                # own rows (blocks 0..7 thanks to the permutation): w factors,
                # transpose to one free-dim row — no DMA (a DMA here starves
                # behind the bulk input stream)
                nc.scalar.activation(
                    wcol_sb[:, :, :], sd_sb[:, 0:IB, 0:8:2],
                    Act.Exp, scale=1.0 - ALPHA, bias=scbias[:, :],
                )
                wct_ps = pswide.tile([32, 128], f16, tag="wctps", bufs=1)
                nc.tensor.transpose(
                    wct_ps, wcol_sb[:, :, :].rearrange("p a b -> p (a b)"),
                    ident_sb[:, :],
                )
                nc.scalar.activation(wct_sb[:, :], wct_ps[:, :], Act.Copy)
                # bounce w through DRAM (partition-major -> head-major) on
                # the scalar engine's HWDGE queue, right after the wct copy
                nc.scalar.dma_start(
                    out=w4_d.rearrange("h b i -> b h i"), in_=wct_sb[:, :]
                )
                for h in range(HPC):
                    emit_wbc(h)
                ps01[0] = alloc_head_psums(0)
                ps01[1] = alloc_head_psums(1)
            if nb > IB:
                emit_ready_quads(nb)
        # drain leftovers h-major so each head's epilogue overlaps the next
        # head's remaining quads
        leftover = phase1[emitted[0]:]
        for hh in range(2):
            for q, h in leftover:
                if h == hh:
                    emit_quad(h, 4 * q, ps01[h])
            emit_epilogue(hh, ps01[hh])
        pswide.release()

        # ---- phase 2: heads 2,3 reuse the released accumulator banks ----
        ps23 = [alloc_head_psums(2), alloc_head_psums(3)]
        for q in range(NQUAD):
            emit_quad(2, 4 * q, ps23[0])
        emit_epilogue(2, ps23[0])
        for q in range(NQUAD):
            emit_quad(3, 4 * q, ps23[1])
        emit_epilogue(3, ps23[1], split=True)
    nc.finalize()
    return nc


def _get_nc():
    if "nc" not in _CACHE:
        _CACHE["nc"] = _build()
    return _CACHE["nc"]


def _prepare_in_maps(x, adj, W, a):
    x = np.asarray(x, np.float32)
    adj = np.asarray(adj, np.float32)
    W = np.asarray(W, np.float32)
    a = np.asarray(a, np.float32)
    xT = np.ascontiguousarray(x.T)
    adjT = np.ascontiguousarray(adj.T)
    all_rows = np.arange(N)
    in_maps = []
    for c in range(NCORES):
        hg, rg = divmod(c, RG)
        own = np.arange(rg * RPC, (rg + 1) * RPC)
        perm = np.concatenate([own, np.delete(all_rows, own)])
        xt = xT[:, perm]
        xhi = xt.astype(np.float16)
        xlo = (xt - xhi.astype(np.float32)).astype(np.float16)
        heads = [hg * HPC + h for h in range(HPC)]
        wsd = np.stack(
            sum([[W[gh] @ a[gh, :HID], W[gh] @ a[gh, HID:]] for gh in heads], []),
            axis=1,
        ).astype(np.float32)  # [F, 8] cols (h0 s, h0 d, h1 s, ...)
        wsdh = wsd.astype(np.float16)
        wsdl = (wsd - wsdh.astype(np.float32)).astype(np.float16)
        wext = np.concatenate(
            [W[gh] for gh in heads] + [wsdh, wsdl], axis=1
        ).astype(np.float16)  # [F, 272]
        adjt_c = (adjT[perm][:, own] * 60000.0).astype(np.float16)

        def swz(m):
            # [K*128, M] -> [128, K, M]: partition-major so each SBUF
            # partition reads one contiguous DRAM run
            k = m.shape[0] // 128
            return np.ascontiguousarray(
                m.reshape(k, 128, m.shape[1]).transpose(1, 0, 2)
            )

        in_maps.append({
            "xhi": swz(xhi),
            "xlo": swz(xlo),
            "wext": swz(wext),
            "adjt": swz(adjt_c),
        })
    return in_maps


def _assemble(results):
    full = np.empty((N, H * HID), np.float32)
    for c in range(NCORES):
        hg, rg = divmod(c, RG)
        o = results[c]["out"]  # [HPC, 128, IB, HID] partition-major
        o = o.transpose(0, 2, 1, 3).reshape(HPC, RPC, HID)
        full[rg * RPC:(rg + 1) * RPC, hg * HPC * HID:(hg + 1) * HPC * HID] = (
            o.transpose(1, 0, 2).reshape(RPC, HPC * HID)
        )
    return full


def _run(in_maps, **kw):
    return run_bass_kernel_spmd(_get_nc(), in_maps, list(range(NCORES)), **kw)


def kernel(x, adj, W, a):
    in_maps = _prepare_in_maps(x, adj, W, a)
    res = _run(in_maps)
    return _assemble(res.results)
